# revision 3
# baseline (speedup 1.0000x reference)
"""Trainium2 Bass kernel for nn_Decoder_15539191677793 (scatter_memory).

Problem: B=128 images of 512x512; each image accumulates 1024 Gaussian-PSF
6x6 patches (integrated-erf profile) at fractional centers given by z.

Strategy (8 NeuronCores, data-parallel on batch: 16 images/core):
  Host: bucket each image's spots by (row-tile m in 0..3 [128 rows],
  col-band c in 0..1 [256 cols]); spots straddling a boundary are duplicated
  into both buckets; each bucket computes only its own window so the split is
  exact. Capacity 256 slots/bucket (mean ~136, +11 sigma); padded slots use
  x0=y0=-1e4 whose erf edge-differences vanish identically.

  Device per (image, bucket, 128-spot block):
    ACT: edge CDFs via one erf op per axis with per-partition bias:
         E[p, e] = erf(e*inv_alpha + bias[p]),  bias = (win0 - 0.5 - x0)*inv_alpha
    DVE: profile values are adjacent edge differences (batched STT over all
         16 blocks of an image); x-side scaled by 250 = 0.25*eta*N0*texp.
    PE : one-hot-free scatter: out[128 rows, 256 cols] accumulates
         Wx^T @ Ry over spot blocks (float32r matmuls, full rate at N=256).
    DMA: PSUM tile -> its (rows, cols) window of the output image in HBM.

  The 6x6 window mask of the reference is dropped: outside the patch the
  erf tails are < ~1e-4 of the output scale (absmax-relative ~2e-7).
"""
import numpy as np

NX, NY = 512, 512
PATCH_HW = 3
P = 2 * PATCH_HW                      # patch side = 6
SIGMA, TEXP, ETA, N0 = 0.92, 1.0, 1.0, 1000.0
ALPHA = float(np.sqrt(np.float32(2.0)) * np.float32(SIGMA))
INV_ALPHA = 1.0 / ALPHA
SCALE = 0.25 * ETA * N0 * TEXP        # the two 0.5s from lx, ly folded with i0

N_CORES = 8
IMG_PER_CORE = 16
N_MTILES = 4                          # row tiles of 128
N_CBANDS = 2                          # col bands of 256
N_BUCKETS = N_MTILES * N_CBANDS
KCAP = 256                            # spot slots per bucket (2 K-blocks of 128)
NKB = KCAP // 128
SLOTS = IMG_PER_CORE * N_BUCKETS * NKB   # columns in XB/YB = 256
PAD_VAL = -1.0e4

_PROGRAM = None


def _build_program():
    import concourse.bacc as bacc
    import concourse.mybir as mybir
    import concourse.tile as tile

    f32 = mybir.dt.float32
    f32r = mybir.dt.float32r
    Alu = mybir.AluOpType
    Erf = mybir.ActivationFunctionType.Erf

    nc = bacc.Bacc("TRN2", target_bir_lowering=False, debug=False)
    xb_d = nc.dram_tensor("xb", [128, SLOTS], f32, kind="ExternalInput")
    yb_d = nc.dram_tensor("yb", [128, SLOTS], f32, kind="ExternalInput")
    bx_d = nc.dram_tensor("basex", [128, SLOTS], f32, kind="ExternalInput")
    by_d = nc.dram_tensor("basey", [128, SLOTS], f32, kind="ExternalInput")
    iox_d = nc.dram_tensor("iox", [128, 129], f32, kind="ExternalInput")
    ioy_d = nc.dram_tensor("ioy", [128, 257], f32, kind="ExternalInput")
    mu_d = nc.dram_tensor("mu", [IMG_PER_CORE, NX, NY], f32, kind="ExternalOutput")

    with tile.TileContext(nc) as tc:
        with (
            tc.tile_pool(name="const", bufs=1) as cpool,
            tc.tile_pool(name="work", bufs=2) as wpool,
            tc.tile_pool(name="psum", bufs=4, space="PSUM") as ppool,
        ):
            xb = cpool.tile([128, SLOTS], f32)
            yb = cpool.tile([128, SLOTS], f32)
            bxc = cpool.tile([128, SLOTS], f32)
            byc = cpool.tile([128, SLOTS], f32)
            iox = cpool.tile([128, 129], f32)
            ioy = cpool.tile([128, 257], f32)
            nc.sync.dma_start(xb[:], xb_d.ap())
            nc.sync.dma_start(yb[:], yb_d.ap())
            nc.sync.dma_start(bxc[:], bx_d.ap())
            nc.sync.dma_start(byc[:], by_d.ap())
            nc.sync.dma_start(iox[:], iox_d.ap())
            nc.sync.dma_start(ioy[:], ioy_d.ap())

            # bias[p, j] = (base_j - 0.5 - coord[p, j]) * inv_alpha, all slots at once.
            biasx = cpool.tile([128, SLOTS], f32)
            biasy = cpool.tile([128, SLOTS], f32)
            nc.vector.scalar_tensor_tensor(
                biasx[:], xb[:], -INV_ALPHA, bxc[:], Alu.mult, Alu.add
            )
            nc.vector.scalar_tensor_tensor(
                biasy[:], yb[:], -INV_ALPHA, byc[:], Alu.mult, Alu.add
            )

            NKT = N_BUCKETS * NKB  # 16 K-block tiles per image
            for img in range(IMG_PER_CORE):
                ex = wpool.tile([128, NKT, 129], f32, tag="ex")
                ey = wpool.tile([128, NKT, 257], f32, tag="ey")
                for t in range(NKT):
                    j = img * NKT + t
                    nc.scalar.activation(
                        ex[:, t], iox[:], Erf, bias=biasx[:, j : j + 1],
                        scale=INV_ALPHA,
                    )
                    nc.scalar.activation(
                        ey[:, t], ioy[:], Erf, bias=biasy[:, j : j + 1],
                        scale=INV_ALPHA,
                    )
                # Batched diffs over all 16 tiles.
                wx = wpool.tile([128, NKT, 128], f32r, tag="wx")
                ry = wpool.tile([128, NKT, 256], f32r, tag="ry")
                nc.vector.scalar_tensor_tensor(
                    wx[:], ex[:, :, 1:], 1.0, ex[:, :, :128], Alu.mult, Alu.subtract
                )
                nc.vector.scalar_tensor_tensor(
                    ry[:], ey[:, :, 1:], 1.0, ey[:, :, :256], Alu.mult, Alu.subtract
                )
                for b in range(N_BUCKETS):
                    m, c = b // N_CBANDS, b % N_CBANDS
                    acc = ppool.tile([128, 256], f32, tag="acc")
                    for kb in range(NKB):
                        t = b * NKB + kb
                        nc.tensor.matmul(
                            acc[:],
                            wx[:, t],
                            ry[:, t],
                            start=(kb == 0),
                            stop=(kb == NKB - 1),
                        )
                    # PSUM -> SBUF evacuation doubles as the 0.25*i0 scaling.
                    out_t = wpool.tile([128, 256], f32, tag="out")
                    nc.vector.tensor_scalar_mul(out_t[:], acc[:], float(SCALE))
                    nc.sync.dma_start(
                        mu_d.ap()[img, 128 * m : 128 * (m + 1), 256 * c : 256 * (c + 1)],
                        out_t[:],
                    )
    nc.finalize()
    return nc


def _host_prep(z):
    """Bucket + pad spots for all cores. Returns in_maps list."""
    B = z.shape[0]
    S = z.shape[1] // 2
    zz = z.reshape(B, 2, S)
    x0a, y0a = zz[:, 0, :], zz[:, 1, :]
    patchx = np.round(x0a).astype(np.int32) - PATCH_HW
    patchy = np.round(y0a).astype(np.int32) - PATCH_HW
    valid = (
        (patchx >= 0) & (patchx < NX - P) & (patchy >= 0) & (patchy < NY - P)
    )

    iox = np.broadcast_to(np.arange(129, dtype=np.float32), (128, 129)).copy()
    ioy = np.broadcast_to(np.arange(257, dtype=np.float32), (128, 257)).copy()

    in_maps = []
    for core in range(N_CORES):
        XB = np.full((128, SLOTS), PAD_VAL, np.float32)
        YB = np.full((128, SLOTS), PAD_VAL, np.float32)
        BX = np.zeros((128, SLOTS), np.float32)
        BY = np.zeros((128, SLOTS), np.float32)
        for li in range(IMG_PER_CORE):
            bimg = core * IMG_PER_CORE + li
            px, py = patchx[bimg], patchy[bimg]
            x0, y0 = x0a[bimg], y0a[bimg]
            v = valid[bimg]
            for m in range(N_MTILES):
                selm = v & (px >= 128 * m - (P - 1)) & (px < 128 * (m + 1))
                for c in range(N_CBANDS):
                    sel = selm & (py >= 256 * c - (P - 1)) & (py < 256 * (c + 1))
                    idx = np.nonzero(sel)[0]
                    n = idx.size
                    if n > KCAP:
                        raise RuntimeError(f"bucket overflow: {n} > {KCAP}")
                    b = m * N_CBANDS + c
                    j0 = li * N_BUCKETS * NKB + b * NKB
                    xs = np.full(KCAP, PAD_VAL, np.float32)
                    ys = np.full(KCAP, PAD_VAL, np.float32)
                    xs[:n] = x0[idx]
                    ys[:n] = y0[idx]
                    XB[:, j0] = xs[:128]
                    XB[:, j0 + 1] = xs[128:]
                    YB[:, j0] = ys[:128]
                    YB[:, j0 + 1] = ys[128:]
                    BX[:, j0 : j0 + 2] = (128.0 * m - 0.5) * INV_ALPHA
                    BY[:, j0 : j0 + 2] = (256.0 * c - 0.5) * INV_ALPHA
        in_maps.append(
            {"xb": XB, "yb": YB, "basex": BX, "basey": BY, "iox": iox, "ioy": ioy}
        )
    return in_maps


def kernel(z: np.ndarray) -> np.ndarray:
    global _PROGRAM
    from concourse.bass_utils import run_bass_kernel_spmd

    if _PROGRAM is None:
        _PROGRAM = _build_program()
    nc = _PROGRAM
    z = np.asarray(z, np.float32)
    in_maps = _host_prep(z)
    res = run_bass_kernel_spmd(nc, in_maps, list(range(N_CORES)))
    mu = np.concatenate([r["mu"] for r in res.results], axis=0)
    return mu.reshape(z.shape[0], 1, NX, NY)
